# revision 1
# baseline (speedup 1.0000x reference)
"""Trainium2 Bass kernel for the 3-layer Clifford (Cl(3,0)) geometric-product MLP.

Math: y[b,o,k] = sum_{i,p,q} x[b,i,p] * w[o,i,q] * C[p,q,k] with the Cayley
table C of Cl(3,0): C[p,q,p^q] = s(p,q) in {+1,-1}, zero elsewhere. This makes
each layer an ordinary dense matmul  Y = X @ Wtil  with X: [B, 4096] (feature
f_in = p*512 + i, blade-major) and Wtil: [4096, 4096] whose (p,k) 512x512 block
is s(p, p^k) * w[:,:,p^k]^T.  Wtil is never materialized: each 128x128 lhsT
tile is an access-pattern slice of the 8 transposed weight slices wT[q][i,o],
and the sign is folded in by choosing the positive or negated copy of the
moving activation operand.

Distribution: data-parallel over batch, 8 cores x 256 rows; weights replicated.
Activations stay on-chip between layers in feature-major layout [f, b], so each
layer's PSUM output [o_tile, b] is directly the next layer's rhs operand.

Precision: matmuls run in float32r (fp32 rounded to 11-bit mantissa, full-rate
on the PE at free dim >= 256, vs 4x slower for plain fp32). Inputs are
RNE-rounded on the host; inter-layer activations are rounded by the DVE copies
that move PSUM->SBUF. Accumulation is fp32 in PSUM and the stored outputs are
full fp32, so per-layer relative error is ~1e-4.
"""

import numpy as np

import concourse.bacc as bacc
import concourse.mybir as mybir
import concourse.tile as tile
from concourse.bass_utils import run_bass_kernel_spmd

B, D, NB = 2048, 512, 8
NCORES = 8
BS = B // NCORES           # 256 batch rows per core
F = D * NB                 # 4096 features per layer
FCH = F // 128             # 32 feature chunks of 128 partitions


def _sign_table():
    """SGN[p,k] = sign of e_p * e_{p^k} in Cl(3,0) (signature +,+,+)."""
    sgn = np.zeros((NB, NB), np.float32)
    for p in range(NB):
        for k in range(NB):
            q = p ^ k
            swaps = 0
            t = p >> 1
            while t:
                swaps += bin(t & q).count("1")
                t >>= 1
            sgn[p, k] = -1.0 if (swaps & 1) else 1.0
    return sgn


def _round_f32r(x):
    """Round-to-nearest-even fp32 -> fp32r (11-bit mantissa, low 12 bits 0)."""
    u = np.ascontiguousarray(x).view(np.uint32)
    lsb = (u >> np.uint32(12)) & np.uint32(1)
    u = u + np.uint32(0x7FF) + lsb
    u = u & np.uint32(0xFFFFF000)
    return u.view(np.float32)


def _build():
    nc = bacc.Bacc("TRN2", target_bir_lowering=False, debug=False)
    f32r, f32 = mybir.dt.float32r, mybir.dt.float32

    x_d = nc.dram_tensor("x", [128, FCH, BS], f32r, kind="ExternalInput")
    w_ds = [
        nc.dram_tensor(f"w{l + 1}", [4, 128, 4, 8, 128], f32r, kind="ExternalInput")
        for l in range(3)
    ]
    y_d = nc.dram_tensor("y", [3, FCH, 128, BS], f32, kind="ExternalOutput")

    SGN = _sign_table()

    with tile.TileContext(nc) as tc:
        with (
            tc.tile_pool(name="act", bufs=4) as act_pool,
            tc.tile_pool(name="wq", bufs=4) as w_pool,
            tc.tile_pool(name="out", bufs=4) as out_pool,
            tc.tile_pool(name="ps", bufs=8, space="PSUM") as ps_pool,
        ):
            # layer-1 activations: x (DMA, host-prerounded) and its negation
            a_pos = act_pool.tile([128, FCH, BS], f32r, tag="act")
            for c in range(4):
                nc.sync.dma_start(a_pos[:, c * 8:(c + 1) * 8, :], x_d[:, c * 8:(c + 1) * 8, :])
            a_neg = act_pool.tile([128, FCH, BS], f32r, tag="act")
            for c in range(4):
                nc.vector.tensor_scalar_mul(
                    a_neg[:, c * 8:(c + 1) * 8, :], a_pos[:, c * 8:(c + 1) * 8, :], -1.0
                )

            for l in range(3):
                if l < 2:
                    a_out_pos = act_pool.tile([128, FCH, BS], f32r, tag="act")
                    a_out_neg = act_pool.tile([128, FCH, BS], f32r, tag="act")
                # o-quarters of the weight stream in as the previous quarter dies,
                # which also prefetches layer l+1 while layer l computes
                for ot in range(4):
                    wq = w_pool.tile([128, 4, 8, 128], f32r, tag="wq")
                    nc.sync.dma_start(wq[:], w_ds[l][ot])
                    for k in range(8):
                        ft = k * 4 + ot  # f_out chunk, f_out = k*512 + ot*128 + o_local
                        ps = ps_pool.tile([128, BS], f32, tag="ps")
                        mm = 0
                        for p in range(8):
                            a_s = a_pos if SGN[p, k] > 0 else a_neg
                            q = p ^ k
                            for io in range(4):
                                nc.tensor.matmul(
                                    ps[:],
                                    wq[:, io, q, :],            # lhsT [128 i, 128 o]
                                    a_s[:, p * 4 + io, :],      # rhs  [128 i, 256 b]
                                    start=(mm == 0),
                                    stop=(mm == 31),
                                )
                                mm += 1
                        o_t = out_pool.tile([128, BS], f32, tag="out")
                        nc.vector.tensor_copy(o_t[:], ps[:])
                        nc.sync.dma_start(y_d[l, ft], o_t[:])
                        if l < 2:
                            nc.vector.tensor_copy(a_out_pos[:, ft, :], ps[:])
                            nc.vector.tensor_scalar_mul(a_out_neg[:, ft, :], ps[:], -1.0)
                if l < 2:
                    a_pos, a_neg = a_out_pos, a_out_neg
    nc.compile()
    return nc


def _prep_inputs(x, w1, w2, w3):
    """Full inputs -> per-core in_maps (numpy, f32r-rounded, device layouts)."""
    # activations: [128 part, 32 fch, 256 b] with f_in = p*512 + i = fch*128 + part
    in_maps = []
    w_arrs = {}
    for name, w in (("w1", w1), ("w2", w2), ("w3", w3)):
        # W[ot, part, io, q, oc] = w[ot*128+oc, io*128+part, q]
        wt = np.asarray(w).transpose(1, 2, 0)            # [i, q, o]
        v = wt.reshape(4, 128, 8, 4, 128)                 # [io, part, q, ot, oc]
        v = np.ascontiguousarray(v.transpose(3, 1, 0, 2, 4))  # [ot, part, io, q, oc]
        w_arrs[name] = _round_f32r(v)
    xa = np.asarray(x)
    for c in range(NCORES):
        xs = xa[c * BS:(c + 1) * BS]                      # [256, 512, 8]
        xt = xs.transpose(2, 1, 0).reshape(FCH, 128, BS)  # [fch, part, b]
        xt = np.ascontiguousarray(xt.transpose(1, 0, 2))  # [128, fch, b]
        m = {"x": _round_f32r(xt)}
        m.update(w_arrs)
        in_maps.append(m)
    return in_maps


def _gather_output(x, results):
    out = np.empty((4, B, D, NB), dtype=np.float32)
    out[0] = np.asarray(x, dtype=np.float32)
    for c in range(NCORES):
        y = results[c]["y"]                               # [3, 32, 128, 256]
        for l in range(3):
            a = y[l].reshape(8, 4, 128, BS)               # [k, ot, part(o_local), b]
            out[l + 1, c * BS:(c + 1) * BS] = (
                a.transpose(3, 1, 2, 0).reshape(BS, D, NB)
            )
    return out


_NC = None


def _get_nc():
    global _NC
    if _NC is None:
        _NC = _build()
    return _NC


def kernel(x, w1, w2, w3):
    nc = _get_nc()
    in_maps = _prep_inputs(x, w1, w2, w3)
    res = run_bass_kernel_spmd(nc, in_maps, core_ids=list(range(NCORES)))
    return _gather_output(x, res.results)


# revision 9
# speedup vs baseline: 334.4235x; 334.4235x over previous
"""Trainium2 Bass kernel for the 3-layer Clifford (Cl(3,0)) geometric-product MLP.

Math: y[b,o,k] = sum_{i,p,q} x[b,i,p] * w[o,i,q] * C[p,q,k] with the Cayley
table C of Cl(3,0). Cl(3,0) is isomorphic to M2(C) via the Pauli matrices
(e_j -> sigma_j), so the geometric product is a 2x2 complex matrix product --
32 real multiplies per blade-pair instead of the 64 of the dense Cayley
contraction. Each layer becomes Y_hat = sum_i X_hat[b,i] @ W_hat[o,i] over
2x2 complex matrices, evaluated as a dense matmul over features
f = h*512 + i, where h in [0,8) indexes the real coordinates (row r, col c,
re/im rho) of the matrix representation.

The blade<->matrix transforms are 2-nonzero-per-row linear maps: the forward
transform is folded into host-side input/weight preparation, layers chain in
the transformed domain directly, and the inverse transform (needed because
every layer's activation is part of the output) is one DVE add/sub per output
chunk: y_k = h_a +- h_b (the 1/2 scale is folded into the layer-1 weights).

Signs of the complex arithmetic (-Xim*Wim in the real part) are folded in by
keeping a negated copy of the activations and selecting it per accumulation
term. Weights are streamed in o-column quarters so the next quarter (and the
next layer) prefetches while the previous one computes.

Distribution: data-parallel over batch, 8 cores x 256 rows; weights replicated.
Activations stay on-chip between layers in feature-major layout [f, b]; each
layer's PSUM output [o_tile, b] is directly the next layer's rhs operand.

Precision: matmuls run in float32r (fp32 rounded to 11-bit mantissa, full PE
rate at moving free dim >= 256; plain fp32 is 4x slower). Inputs are
RNE-rounded on the host; inter-layer activations are rounded by the DVE copies
out of PSUM. Accumulation is fp32 in PSUM; outputs are fp32.
"""

import numpy as np

import concourse.bacc as bacc
import concourse.mybir as mybir
import concourse.tile as tile
from concourse.bass_utils import run_bass_kernel_spmd

B, D, NB = 2048, 512, 8
NCORES = 8
BS = B // NCORES           # 256 batch rows per core
F = D * NB                 # 4096 features per layer
FCH = F // 128             # 32 feature chunks of 128 partitions


def _tx_table():
    """TX[h, blade]: blade coords -> M2(C) real coords h = r*4 + c*2 + rho."""
    tx = np.zeros((8, 8), np.float32)

    def put(r, c, rho, blade, s):
        tx[r * 4 + c * 2 + rho, blade] += s

    put(0, 0, 0, 0, 1); put(1, 1, 0, 0, 1)      # 1    -> I
    put(0, 1, 0, 1, 1); put(1, 0, 0, 1, 1)      # e1   -> s1
    put(0, 1, 1, 2, -1); put(1, 0, 1, 2, 1)     # e2   -> s2
    put(0, 0, 0, 4, 1); put(1, 1, 0, 4, -1)     # e3   -> s3
    put(0, 0, 1, 3, 1); put(1, 1, 1, 3, -1)     # e12  -> i*s3
    put(0, 1, 0, 5, -1); put(1, 0, 0, 5, 1)     # e13  -> s1*s3
    put(0, 1, 1, 6, 1); put(1, 0, 1, 6, 1)      # e23  -> i*s1
    put(0, 0, 1, 7, 1); put(1, 1, 1, 7, 1)      # e123 -> i*I
    return tx


# output extraction (2*TI rows): y_blade = h_a (+|-) h_b
_YCOMB = [
    (0, 0, 6, "add"),
    (4, 0, 6, "sub"),
    (1, 2, 4, "add"),
    (5, 4, 2, "sub"),
    (2, 5, 3, "sub"),
    (6, 3, 5, "add"),
    (3, 1, 7, "sub"),
    (7, 1, 7, "add"),
]


def _round_f32r(x):
    """Round-to-nearest-even fp32 -> fp32r (11-bit mantissa, low 12 bits 0)."""
    u = np.ascontiguousarray(x).view(np.uint32)
    lsb = (u >> np.uint32(12)) & np.uint32(1)
    u = u + np.uint32(0x7FF) + lsb
    u = u & np.uint32(0xFFFFF000)
    return u.view(np.float32)


def _build(repeat=1):
    import contextlib

    nc = bacc.Bacc("TRN2", target_bir_lowering=False, debug=False)
    f32r, f32 = mybir.dt.float32r, mybir.dt.float32

    x_d = nc.dram_tensor("x", [128, FCH, BS], f32r, kind="ExternalInput")
    w_ds = [
        nc.dram_tensor(f"w{l + 1}", [4, 128, 4, 8, 128], f32r, kind="ExternalInput")
        for l in range(3)
    ]
    y_d = nc.dram_tensor("y", [3, 4, 128, 8, BS], f32, kind="ExternalOutput")

    with tile.TileContext(nc) as tc:
        with (
            tc.tile_pool(name="act", bufs=4) as act_pool,
            tc.tile_pool(name="wq", bufs=4) as w_pool,
            tc.tile_pool(name="out", bufs=1) as out_pool,
            tc.tile_pool(name="ps", bufs=8, space="PSUM") as ps_pool,
            tc.For_i(0, repeat, 1) if repeat > 1 else contextlib.nullcontext(),
        ):
            a_pos = act_pool.tile([128, FCH, BS], f32r, tag="act")
            for c4 in range(4):
                nc.sync.dma_start(
                    a_pos[:, c4 * 8:(c4 + 1) * 8, :], x_d[:, c4 * 8:(c4 + 1) * 8, :]
                )
            a_neg = act_pool.tile([128, FCH, BS], f32r, tag="act")
            for c4 in range(4):
                nc.vector.tensor_scalar_mul(
                    a_neg[:, c4 * 8:(c4 + 1) * 8, :], a_pos[:, c4 * 8:(c4 + 1) * 8, :], -1.0
                )

            for l in range(3):
                a_out_pos = act_pool.tile([128, FCH, BS], f32r, tag="act")
                if l < 2:
                    a_out_neg = act_pool.tile([128, FCH, BS], f32r, tag="act")
                for ot in range(4):
                    wq = w_pool.tile([128, 4, 8, 128], f32r, tag="wq")
                    nc.scalar.dma_start(wq[:, 0:2, :, :], w_ds[l][ot][:, 0:2, :, :])
                    nc.sync.dma_start(wq[:, 2:4, :, :], w_ds[l][ot][:, 2:4, :, :])
                    for cr in range(4):
                        # pair the r=0 / r=1 output chunks of the same (c, rho)
                        # into one PSUM bank: one matmul serves both (the sign
                        # and the weight tile depend only on (t, c, rho, rho')),
                        # moving free dim 512 amortizes the fp32r weight load
                        cc, rho = cr >> 1, cr & 1
                        ps = ps_pool.tile([128, 2, BS], f32, tag="ps")
                        mm = 0
                        for t in range(2):
                            for rp in range(2):
                                hw = t * 4 + cc * 2 + (rho ^ rp)
                                a_s = a_neg if (rp == 1 and rho == 0) else a_pos
                                a_v = a_s[:].rearrange("p (r f) b -> p r f b", r=2)
                                for io in range(4):
                                    nc.tensor.matmul(
                                        ps[:],
                                        wq[:, io, hw, :],               # lhsT [128 i, 128 o]
                                        a_v[:, :, (t * 2 + rp) * 4 + io, :],  # rhs [128 i, 2 r, 256 b]
                                        start=(mm == 0),
                                        stop=(mm == 15),
                                    )
                                    mm += 1
                        for r in range(2):
                            h_out = r * 4 + cc * 2 + rho
                            nc.vector.tensor_copy(a_out_pos[:, h_out * 4 + ot, :], ps[:, r, :])
                            if l < 2:
                                nc.vector.tensor_scalar_mul(
                                    a_out_neg[:, h_out * 4 + ot, :], ps[:, r, :], -1.0
                                )
                    o_t = out_pool.tile([128, 8, BS], f32, tag="out")
                    for kb, ha, hb, op in _YCOMB:
                        src_a = a_out_pos[:, ha * 4 + ot, :]
                        src_b = a_out_pos[:, hb * 4 + ot, :]
                        if op == "add":
                            nc.vector.tensor_add(o_t[:, kb, :], src_a, src_b)
                        else:
                            nc.vector.tensor_sub(o_t[:, kb, :], src_a, src_b)
                    nc.sync.dma_start(y_d[l, ot], o_t[:])
                if l < 2:
                    a_pos, a_neg = a_out_pos, a_out_neg
    nc.compile()
    return nc


def _prep_inputs(x, w1, w2, w3):
    """Full inputs -> per-core in_maps (numpy, f32r-rounded, device layouts)."""
    tx = _tx_table()
    in_maps = []
    w_arrs = {}
    for idx, (name, w) in enumerate((("w1", w1), ("w2", w2), ("w3", w3))):
        wh = (np.asarray(w, np.float32).reshape(-1, 8) @ tx.T).reshape(D, D, 8)
        if idx == 0:
            wh = wh * np.float32(0.5)  # fold the inverse-transform 1/2 scale
        # quarter layout: W[ot, part, io, hw, oc] = wh[ot*128+oc, io*128+part, hw]
        wt = wh.transpose(1, 2, 0)                            # [i, hw, o]
        v = wt.reshape(4, 128, 8, 4, 128)                     # [io, part, hw, ot, oc]
        v = np.ascontiguousarray(v.transpose(3, 1, 0, 2, 4))  # [ot, part, io, hw, oc]
        w_arrs[name] = _round_f32r(v)
    xa = np.asarray(x, np.float32)
    xh = (xa.reshape(-1, 8) @ tx.T).reshape(B, D, 8)          # [b, i, h]
    for c in range(NCORES):
        xs = xh[c * BS:(c + 1) * BS]                          # [256, 512, 8]
        xt = xs.transpose(2, 1, 0).reshape(FCH, 128, BS)      # [fc=h*4+io, part, b]
        xt = np.ascontiguousarray(xt.transpose(1, 0, 2))      # [128, fc, b]
        m = {"x": _round_f32r(xt)}
        m.update(w_arrs)
        in_maps.append(m)
    return in_maps


def _gather_output(x, results):
    out = np.empty((4, B, D, NB), dtype=np.float32)
    out[0] = np.asarray(x, dtype=np.float32)
    for c in range(NCORES):
        y = results[c]["y"]                                   # [3, 4, 128, 8, 256]
        for l in range(3):
            # y[l]: [ot, oc, kb, b] -> [b, (ot,oc)=o, kb]
            out[l + 1, c * BS:(c + 1) * BS] = (
                y[l].transpose(3, 0, 1, 2).reshape(BS, D, NB)
            )
    return out


_NC = None


def _get_nc():
    global _NC
    if _NC is None:
        _NC = _build()
    return _NC


def kernel(x, w1, w2, w3):
    nc = _get_nc()
    in_maps = _prep_inputs(x, w1, w2, w3)
    res = run_bass_kernel_spmd(nc, in_maps, core_ids=list(range(NCORES)))
    return _gather_output(x, res.results)
